# revision 11
# baseline (speedup 1.0000x reference)
"""AlignmentContrastiveLoss (MrSw) on 8 Trainium2 NeuronCores.

Strategy
--------
align[i,j,r,w] = <im[i,r,:], s[j,w,:]>  with padded regions/words zeroed.
Zeroing the padded rows of `im` and padded words of `s` on the host makes
the matmul output exactly equal to the reference's masked_fill(0) tensor,
so no on-device masking is needed.

Sharding: image batch axis i across 8 cores (16 images/core); s replicated.

Per core, for each word index w (37 of them), the TensorEngine computes
    psum_w[j, (i,r)] = sum_d s[j, w, d] * im[i, r, d]        [128 x 784]
as 8 accumulating K=128 matmuls (stationary = s[:, w, :]^T chunk, moving =
im^T chunk).  Both MrSw reductions then happen in cheap directions:
  - max over w  : running elementwise tensor_max across the 37 psum tiles
  - max over r  : free-dim segmented reduce ([128,16,49] -> [128,16]) per w
  - sum over r  : free-dim segmented reduce of the running max
  - sum over w  : free-dim reduce of the stacked per-w maxes
Output per core is [128 j, 16 i_local] fp32; host transposes and stacks.

The 784-wide moving dim is stored as two bank-aligned halves of 392
valid columns in a [128, 2, 512] PSUM tile, so on the last w the k-loop
runs half-by-half and the serial DVE tail overlaps the final matmuls.
"""

import numpy as np
import ml_dtypes

import concourse.bacc as bacc
import concourse.mybir as mybir
import concourse.tile as tile
from concourse.bass_utils import run_bass_kernel_spmd

B = 128          # batch (images == sentences)
L_IM, L_S, D = 50, 40, 1024
R = L_IM - 1     # 49 regions
W = L_S - 3      # 37 words
NCORES = 8
IPC = B // NCORES            # 16 images per core
N = IPC * R                  # 784 = moving free dim
NH = N // 2                  # 392 = half (8 images x 49 regions)
IH = IPC // 2                # 8
KC = D // 128                # 8 contraction chunks
BF16 = mybir.dt.bfloat16
F32 = mybir.dt.float32
X = mybir.AxisListType.X

_NC_CACHE = None


def _build():
    nc = bacc.Bacc("TRN2", target_bir_lowering=False, debug=False,
                   num_devices=NCORES)
    # sT layout: [dk(128 partitions), w, k, j]  (row-contiguous per partition)
    sT = nc.dram_tensor("sT", [128, W * KC * B], BF16, kind="ExternalInput")
    # imT layout: [dk(128), k, (i,r)]
    imT = nc.dram_tensor("imT", [128, KC * N], BF16, kind="ExternalInput")
    out = nc.dram_tensor("out", [B, IPC], F32, kind="ExternalOutput")

    with tile.TileContext(nc) as tc:
        with (
            tc.tile_pool(name="persist", bufs=1) as persist,
            tc.tile_pool(name="sw", bufs=6) as swpool,
            tc.tile_pool(name="ps", bufs=3, space="PSUM") as pspool,
            tc.tile_pool(name="warm", bufs=1, space="PSUM") as warmpool,
        ):
            # --- startup DMAs -------------------------------------------
            # First matmul is gated on s_w0 + imt chunk 0: those go first,
            # split in halves across queues so fixed DMA latencies overlap.
            s_tiles = [None] * W
            s_tiles[0] = swpool.tile([128, KC * 128], BF16, tag="s_w",
                                     name="s_w0")
            nc.sync.dma_start(s_tiles[0][:, 0:512], sT.ap()[:, 0:512])
            nc.scalar.dma_start(s_tiles[0][:, 512:KC * B],
                                sT.ap()[:, 512:KC * B])

            # im chunks in pairs: fewer, bigger transfers
            imp = [persist.tile([128, 2 * N], BF16, tag=f"imp{p}",
                                name=f"imp{p}") for p in range(KC // 2)]
            nc.gpsimd.dma_start(imp[0][:, 0:N], imT.ap()[:, 0:N])
            nc.sync.dma_start(imp[0][:, N:2 * N], imT.ap()[:, N:2 * N])
            nc.scalar.dma_start(imp[1][:], imT.ap()[:, 2 * N:4 * N])
            nc.gpsimd.dma_start(imp[2][:], imT.ap()[:, 4 * N:6 * N])
            nc.scalar.dma_start(imp[3][:], imT.ap()[:, 6 * N:8 * N])

            def im_k(k, half):      # [128, 392] moving operand slice
                return imp[k // 2][:, (k % 2) * N + half * NH:
                                   (k % 2) * N + (half + 1) * NH]

            # --- PE warm-up ---------------------------------------------
            # HAM unthrottles the PE clock (1.2 -> 2.4 GHz) after ~3.4us of
            # sustained activity; burn dummy matmuls while DMAs land.
            dummy = persist.tile([128, 128], BF16)
            nc.vector.memset(dummy[:], 0)
            warm = warmpool.tile([128, 128], F32)
            for _ in range(38):
                nc.tensor.matmul(warm[:], dummy[:], dummy[:],
                                 start=True, stop=True)

            runmax = persist.tile([128, 2, NH], F32)    # max over w so far
            maxr = persist.tile([128, IPC, W], F32)     # per-w max over r
            term1a = persist.tile([128, IPC], F32)      # partial sum_w maxr

            def reduce_maxr(ps, w, half=None):
                """maxr[:, :, w] = max over r of ps (one half or both)."""
                if half is None:
                    nc.vector.reduce_max(
                        maxr[:, :, w],
                        ps[:, :, 0:NH].rearrange("p h (i r) -> p h i r", r=R),
                        axis=X)
                else:
                    nc.vector.reduce_max(
                        maxr[:, half * IH:(half + 1) * IH, w],
                        ps[:, half, 0:NH].rearrange("p (i r) -> p i r", r=R),
                        axis=X)

            s_issue = [nc.sync, nc.scalar]
            for w in range(W):
                if w > 0:
                    s_tiles[w] = swpool.tile([128, KC * 128], BF16,
                                             tag="s_w", name=f"s_w{w}")
                    s_issue[w % 2].dma_start(
                        s_tiles[w][:], sT.ap()[:, w * KC * B:(w + 1) * KC * B])
                s_w = s_tiles[w]

                ps = pspool.tile([128, 2, 512], F32)
                last = w == W - 1
                # normal w: interleave halves so weight loads are shared;
                # last w: all of half 0 first so its DVE tail overlaps the
                # half-1 matmuls.
                halves_outer = last
                if halves_outer:
                    for half in (0, 1):
                        for k in range(KC):
                            nc.tensor.matmul(ps[:, half, 0:NH],
                                             s_w[:, k * 128:(k + 1) * 128],
                                             im_k(k, half),
                                             start=(k == 0), stop=(k == KC - 1))
                else:
                    for k in range(KC):
                        lhsT = s_w[:, k * 128:(k + 1) * 128]
                        for half in (0, 1):
                            nc.tensor.matmul(ps[:, half, 0:NH],
                                             lhsT, im_k(k, half),
                                             start=(k == 0), stop=(k == KC - 1))

                if last:
                    # ---- half 0 tail (overlaps half-1 matmuls) ----
                    reduce_maxr(ps, w, half=0)
                    term1 = persist.tile([128, IPC], F32)
                    nc.vector.reduce_sum(term1[:, 0:IH],
                                         maxr[:, 0:IH, 32:W], axis=X)
                    nc.vector.tensor_max(runmax[:, 0, :], runmax[:, 0, :],
                                         ps[:, 0, 0:NH])
                    term2 = persist.tile([128, IPC], F32)
                    nc.vector.reduce_sum(
                        term2[:, 0:IH],
                        runmax[:, 0, :].rearrange("p (i r) -> p i r", r=R),
                        axis=X)
                    # ---- half 1 tail ----
                    reduce_maxr(ps, w, half=1)
                    nc.vector.reduce_sum(term1[:, IH:IPC],
                                         maxr[:, IH:IPC, 32:W], axis=X)
                    nc.vector.tensor_max(runmax[:, 1, :], runmax[:, 1, :],
                                         ps[:, 1, 0:NH])
                    nc.vector.reduce_sum(
                        term2[:, IH:IPC],
                        runmax[:, 1, :].rearrange("p (i r) -> p i r", r=R),
                        axis=X)
                    # ---- combine + store ----
                    res = persist.tile([128, IPC], F32)
                    nc.vector.tensor_add(res[:], term1[:], term2[:])
                    nc.vector.tensor_add(res[:], res[:], term1a[:])
                    nc.sync.dma_start(out.ap()[:], res[:])
                else:
                    if w == 0:
                        nc.vector.tensor_copy(runmax[:], ps[:, :, 0:NH])
                    else:
                        nc.vector.tensor_max(runmax[:], runmax[:],
                                             ps[:, :, 0:NH])
                    reduce_maxr(ps, w)
                    if w == 31:
                        # fold w=0..31 of term1 while there is DVE slack
                        nc.vector.reduce_sum(term1a[:], maxr[:, :, 0:32],
                                             axis=X)

    nc.compile()
    return nc


def _get_nc():
    global _NC_CACHE
    if _NC_CACHE is None:
        _NC_CACHE = _build()
    return _NC_CACHE


def kernel(im_set, s_seq, im_len, s_len):
    im_set = np.asarray(im_set, dtype=np.float32)
    s_seq = np.asarray(s_seq, dtype=np.float32)
    im_len = np.asarray(im_len).astype(np.int64)
    s_len = np.asarray(s_len).astype(np.int64)

    im = im_set[:, 1:, :].copy()          # [B, R, D]
    s = s_seq[:, 1:-2, :].copy()          # [B, W, D]
    il = im_len - 1
    sl = s_len - 3
    im *= (np.arange(R)[None, :] < il[:, None])[:, :, None]
    s *= (np.arange(W)[None, :] < sl[:, None])[:, :, None]

    # sT[dk, w, k, j] = s[j, w, k*128+dk]
    sT = (s.transpose(2, 1, 0)                 # [D, W, B]
          .reshape(KC, 128, W, B)              # [k, dk, w, j]
          .transpose(1, 2, 0, 3)               # [dk, w, k, j]
          .reshape(128, W * KC * B)
          .astype(ml_dtypes.bfloat16))

    in_maps = []
    for c in range(NCORES):
        im_c = im[c * IPC:(c + 1) * IPC]       # [IPC, R, D]
        imT = (im_c.reshape(N, D)
               .T                              # [D, N]
               .reshape(KC, 128, N)            # [k, dk, ir]
               .transpose(1, 0, 2)             # [dk, k, ir]
               .reshape(128, KC * N)
               .astype(ml_dtypes.bfloat16))
        in_maps.append({"sT": sT, "imT": np.ascontiguousarray(imT)})

    nc = _get_nc()
    res = run_bass_kernel_spmd(nc, in_maps, core_ids=list(range(NCORES)))

    full = np.empty((B, B), dtype=np.float32)
    for c in range(NCORES):
        full[c * IPC:(c + 1) * IPC, :] = res.results[c]["out"].T
    return full


# revision 14
# speedup vs baseline: 1.4447x; 1.4447x over previous
"""AlignmentContrastiveLoss (MrSw) on 8 Trainium2 NeuronCores.

Strategy
--------
align[i,j,r,w] = <im[i,r,:], s[j,w,:]>  with padded regions/words zeroed.
Zeroing the padded rows of `im` and padded words of `s` on the host makes
the matmul output exactly equal to the reference's masked_fill(0) tensor,
so no on-device masking is needed.

Sharding: image batch axis i across 8 cores (16 images/core); s replicated.

Per core, for each word index w (37 of them), the TensorEngine computes
    psum_w[j, (i,r)] = sum_d s[j, w, d] * im[i, r, d]        [128 x 784]
as 8 accumulating K=128 matmuls (stationary = s[:, w, :]^T chunk, moving =
im^T chunk).  Both MrSw reductions then happen in cheap directions:
  - max over w  : running elementwise tensor_max across the 37 psum tiles
  - max over r  : free-dim segmented reduce ([128,16,49] -> [128,16]) per w
  - sum over r  : free-dim segmented reduce of the running max
  - sum over w  : free-dim reduce of the stacked per-w maxes
Output per core is [128 j, 16 i_local] fp32; host transposes and stacks.

The 784-wide moving dim is stored as two bank-aligned halves of 392
valid columns in a [128, 2, 512] PSUM tile, so on the last w the k-loop
runs half-by-half and the serial DVE tail overlaps the final matmuls.
"""

import numpy as np
import ml_dtypes

import concourse.bacc as bacc
import concourse.mybir as mybir
import concourse.tile as tile
from concourse.bass_utils import run_bass_kernel_spmd

B = 128          # batch (images == sentences)
L_IM, L_S, D = 50, 40, 1024
R = L_IM - 1     # 49 regions
W = L_S - 3      # 37 words
NCORES = 8
IPC = B // NCORES            # 16 images per core
N = IPC * R                  # 784 = moving free dim
NH = N // 2                  # 392 = half (8 images x 49 regions)
IH = IPC // 2                # 8
KC = D // 128                # 8 contraction chunks
BF16 = mybir.dt.bfloat16
F32 = mybir.dt.float32
X = mybir.AxisListType.X

_NC_CACHE = None


def _build():
    nc = bacc.Bacc("TRN2", target_bir_lowering=False, debug=False,
                   num_devices=NCORES)
    # sT layout: [dk(128 partitions), w, k, j]  (row-contiguous per partition)
    sT = nc.dram_tensor("sT", [128, W * KC * B], BF16, kind="ExternalInput")
    # imT layout: [dk(128), k, (i,r)]
    imT = nc.dram_tensor("imT", [128, KC * N], BF16, kind="ExternalInput")
    out = nc.dram_tensor("out", [B, IPC], F32, kind="ExternalOutput")

    with tile.TileContext(nc) as tc:
        with (
            tc.tile_pool(name="persist", bufs=1) as persist,
            tc.tile_pool(name="sw", bufs=6) as swpool,
            tc.tile_pool(name="ps", bufs=3, space="PSUM") as pspool,
            tc.tile_pool(name="warm", bufs=1, space="PSUM") as warmpool,
        ):
            # --- startup DMAs -------------------------------------------
            # First matmul is gated on s_w0 + imt chunk 0: those go first,
            # split in halves across queues so fixed DMA latencies overlap.
            s_tiles = [None] * W
            s_tiles[0] = swpool.tile([128, KC * 128], BF16, tag="s_w",
                                     name="s_w0")
            nc.sync.dma_start(s_tiles[0][:, 0:512], sT.ap()[:, 0:512])
            nc.scalar.dma_start(s_tiles[0][:, 512:KC * B],
                                sT.ap()[:, 512:KC * B])

            # im chunks in pairs: fewer, bigger transfers
            imp = [persist.tile([128, 2 * N], BF16, tag=f"imp{p}",
                                name=f"imp{p}") for p in range(KC // 2)]
            nc.gpsimd.dma_start(imp[0][:, 0:N], imT.ap()[:, 0:N])
            nc.sync.dma_start(imp[0][:, N:2 * N], imT.ap()[:, N:2 * N])
            nc.scalar.dma_start(imp[1][:], imT.ap()[:, 2 * N:4 * N])
            nc.gpsimd.dma_start(imp[2][:], imT.ap()[:, 4 * N:6 * N])
            nc.scalar.dma_start(imp[3][:], imT.ap()[:, 6 * N:8 * N])

            def im_k(k, half):      # [128, 392] moving operand slice
                return imp[k // 2][:, (k % 2) * N + half * NH:
                                   (k % 2) * N + (half + 1) * NH]

            # --- PE warm-up ---------------------------------------------
            # HAM unthrottles the PE clock (1.2 -> 2.4 GHz) after ~3.4us of
            # sustained activity; burn dummy matmuls while DMAs land.
            dummy = persist.tile([128, 128], BF16)
            nc.vector.memset(dummy[:], 0)
            warm = warmpool.tile([128, 128], F32)
            for _ in range(38):
                nc.tensor.matmul(warm[:], dummy[:], dummy[:],
                                 start=True, stop=True)

            runmax = persist.tile([128, 2, NH], F32)    # max over w so far
            maxr = persist.tile([128, IPC, W], F32)     # per-w max over r
            term1a = persist.tile([128, IPC], F32)      # partial sum_w maxr

            def reduce_maxr(ps, w, half=None):
                """maxr[:, :, w] = max over r of ps (one half or both)."""
                if half is None:
                    nc.vector.reduce_max(
                        maxr[:, :, w],
                        ps[:, :, 0:NH].rearrange("p h (i r) -> p h i r", r=R),
                        axis=X)
                else:
                    nc.vector.reduce_max(
                        maxr[:, half * IH:(half + 1) * IH, w],
                        ps[:, half, 0:NH].rearrange("p (i r) -> p i r", r=R),
                        axis=X)

            s_issue = [nc.sync, nc.scalar]
            for w in range(W):
                if w > 0:
                    s_tiles[w] = swpool.tile([128, KC * 128], BF16,
                                             tag="s_w", name=f"s_w{w}")
                    s_issue[w % 2].dma_start(
                        s_tiles[w][:], sT.ap()[:, w * KC * B:(w + 1) * KC * B])
                s_w = s_tiles[w]

                last = w == W - 1
                if last:
                    # Two separate PSUM tiles so the half-0 DVE chain only
                    # depends on half-0's matmuls and overlaps half-1's.
                    psh = [pspool.tile([128, 2, 512], F32, tag="ps",
                                       name=f"psl{h}") for h in (0, 1)]
                    term1 = persist.tile([128, IPC], F32)
                    term2 = persist.tile([128, IPC], F32)
                    for half in (0, 1):
                        for k in range(KC):
                            nc.tensor.matmul(psh[half][:, 0, 0:NH],
                                             s_w[:, k * 128:(k + 1) * 128],
                                             im_k(k, half),
                                             start=(k == 0), stop=(k == KC - 1))
                        lo, hi = half * IH, (half + 1) * IH
                        nc.vector.reduce_max(
                            maxr[:, lo:hi, w],
                            psh[half][:, 0, 0:NH].rearrange("p (i r) -> p i r",
                                                         r=R),
                            axis=X)
                        nc.vector.reduce_sum(term1[:, lo:hi],
                                             maxr[:, lo:hi, 32:W], axis=X)
                        nc.vector.tensor_max(runmax[:, half, :],
                                             runmax[:, half, :],
                                             psh[half][:, 0, 0:NH])
                        nc.vector.reduce_sum(
                            term2[:, lo:hi],
                            runmax[:, half, :].rearrange("p (i r) -> p i r",
                                                         r=R),
                            axis=X)
                    # ---- combine + store ----
                    res = persist.tile([128, IPC], F32)
                    nc.vector.tensor_add(res[:], term1[:], term2[:])
                    nc.vector.tensor_add(res[:], res[:], term1a[:])
                    nc.sync.dma_start(out.ap()[:], res[:])
                else:
                    ps = pspool.tile([128, 2, 512], F32)
                    for k in range(KC):
                        lhsT = s_w[:, k * 128:(k + 1) * 128]
                        for half in (0, 1):
                            nc.tensor.matmul(ps[:, half, 0:NH],
                                             lhsT, im_k(k, half),
                                             start=(k == 0), stop=(k == KC - 1))
                    if w == 0:
                        nc.vector.tensor_copy(runmax[:], ps[:, :, 0:NH])
                    else:
                        nc.vector.tensor_max(runmax[:], runmax[:],
                                             ps[:, :, 0:NH])
                    reduce_maxr(ps, w)
                    if w == 31:
                        # fold w=0..31 of term1 while there is DVE slack
                        nc.vector.reduce_sum(term1a[:], maxr[:, :, 0:32],
                                             axis=X)

    nc.compile()
    return nc


def _get_nc():
    global _NC_CACHE
    if _NC_CACHE is None:
        _NC_CACHE = _build()
    return _NC_CACHE


def kernel(im_set, s_seq, im_len, s_len):
    im_set = np.asarray(im_set, dtype=np.float32)
    s_seq = np.asarray(s_seq, dtype=np.float32)
    im_len = np.asarray(im_len).astype(np.int64)
    s_len = np.asarray(s_len).astype(np.int64)

    im = im_set[:, 1:, :].copy()          # [B, R, D]
    s = s_seq[:, 1:-2, :].copy()          # [B, W, D]
    il = im_len - 1
    sl = s_len - 3
    im *= (np.arange(R)[None, :] < il[:, None])[:, :, None]
    s *= (np.arange(W)[None, :] < sl[:, None])[:, :, None]

    # sT[dk, w, k, j] = s[j, w, k*128+dk]
    sT = (s.transpose(2, 1, 0)                 # [D, W, B]
          .reshape(KC, 128, W, B)              # [k, dk, w, j]
          .transpose(1, 2, 0, 3)               # [dk, w, k, j]
          .reshape(128, W * KC * B)
          .astype(ml_dtypes.bfloat16))

    in_maps = []
    for c in range(NCORES):
        im_c = im[c * IPC:(c + 1) * IPC]       # [IPC, R, D]
        imT = (im_c.reshape(N, D)
               .T                              # [D, N]
               .reshape(KC, 128, N)            # [k, dk, ir]
               .transpose(1, 0, 2)             # [dk, k, ir]
               .reshape(128, KC * N)
               .astype(ml_dtypes.bfloat16))
        in_maps.append({"sT": sT, "imT": np.ascontiguousarray(imT)})

    nc = _get_nc()
    res = run_bass_kernel_spmd(nc, in_maps, core_ids=list(range(NCORES)))

    full = np.empty((B, B), dtype=np.float32)
    for c in range(NCORES):
        full[c * IPC:(c + 1) * IPC, :] = res.results[c]["out"].T
    return full
